# revision 23
# baseline (speedup 1.0000x reference)
"""Llama block (single-token decode) on 8 TRN2 NeuronCores, tensor-parallel.

Sharding (per core c of 8):
  - heads 4c..4c+3: w_q/w_k/w_v column shards [4096, 512], KV cache [4096, 4, 128]
  - w_o row shard [512, 4096] -> partial attn output, AllReduce'd on device
  - w_ff1 column shard [4096, 1376->1408pad], w_ff2 row shard [1376->1408pad, 4096]
  - per-core FFN partials summed on host (row-sharded output unshard)

Perf design vs the f32 baseline:
  - All large tensors cast to bf16 ON HOST and repacked tile-major so every
    streaming DMA is a contiguous ~1 MB transfer (descriptor-efficient).
  - One HWDGE queue (nc.sync) carries the 43 stream chunks in exact
    consumption order: wq, wk, kc, wv, vc, wo, ff1, ff2.  All small or
    dependency-carrying DMAs ride SWDGE (nc.gpsimd) so the stream never
    blocks behind the AllReduce.
  - Softmax uses a constant shift (exp(s - B), B=20) instead of the true
    max: mathematically identical after normalization, and it removes the
    all-chunk barrier - each KV chunk flows scores -> exp -> o@V pipelined.
  - Scores are batched DVE ops (one mult + one segmented reduce per chunk).
  - PSUM accumulation groups are packed three-per-bank at base partitions
    0/32/64 (qkv in one bank, ff2's 8 strips in 3 banks, single pass).
  - PE warm-up spinner matmuls bridge the AllReduce gap so the HAM clock
    gate stays open for the ff1/ff2 GEMVs.
  - ACT table loads (sqrt/exp/sigmoid) are prefetched by dummy ops placed
    right after the previous function's last real use.
  - AllReduce payload is bf16 (8 KB) and overlaps the ff weight stream.

On-chip layout convention: a length-4096 vector is held as [128, 32] "cols"
(element (p, t) = vec[t*128 + p]) so vector tiles feed matmul lhsT directly.
"""

import math
import sys

sys.path.insert(0, "/opt/trn_rl_repo")

import numpy as np
import ml_dtypes

import concourse.bass as bass
import concourse.tile as tile
from concourse import bacc, mybir
from concourse.bass_utils import run_bass_kernel_spmd
from concourse import bass_isa

F32 = mybir.dt.float32
BF16 = mybir.dt.bfloat16
AF = mybir.ActivationFunctionType
ALU = mybir.AluOpType
AX = mybir.AxisListType

H = 4096
NH = 32
HD = 128
INTERM = 11008
EPS = 1e-6
CORES = 8
HPC = NH // CORES  # 4 heads per core
QC = HPC * HD  # 512 qkv cols per core
FFC = INTERM // CORES  # 1376 ff cols per core
FFCP = 1408  # padded to 11 * 128
FFKT = FFCP // 128  # 11 contraction tiles for ff2
KT = H // 128  # 32 contraction tiles
SCALE = 1.0 / math.sqrt(HD)
EXP_SHIFT = -20.0  # constant softmax shift; |scores| << 20 for this model
NP_BF16 = ml_dtypes.bfloat16
N_WARM = 96  # PE warm-up spinner matmuls bridging the AllGather

_BUILD_CACHE = {}


def _build(pos: int):
    if pos in _BUILD_CACHE:
        return _BUILD_CACHE[pos]

    n_s = pos + 1
    n_tiles = (n_s + 127) // 128  # s-tiles to attend over
    rem = n_s - (n_tiles - 1) * 128  # rows in last s-tile (1..128)
    pos_tile = pos // 128
    pos_row = pos % 128
    n_kv_chunks = (n_tiles + 7) // 8

    nc = bacc.Bacc("TRN2", target_bir_lowering=False, debug=False, num_devices=CORES)

    x_in = nc.dram_tensor("x_cols", [128, KT], F32, kind="ExternalInput")
    an_in = nc.dram_tensor("an_cols", [128, KT], F32, kind="ExternalInput")
    fn_in = nc.dram_tensor("fn_cols", [128, KT], F32, kind="ExternalInput")
    rope_in = nc.dram_tensor("rope_tbl", [4, QC], F32, kind="ExternalInput")
    eye32_in = nc.dram_tensor("eye32", [32, 32], F32, kind="ExternalInput")
    wq_in = nc.dram_tensor("wq", [4, 128, 8, QC], BF16, kind="ExternalInput")
    wk_in = nc.dram_tensor("wk", [4, 128, 8, QC], BF16, kind="ExternalInput")
    wv_in = nc.dram_tensor("wv", [4, 128, 8, QC], BF16, kind="ExternalInput")
    wo_in = nc.dram_tensor("wo", [4, 128, H], BF16, kind="ExternalInput")
    kc_in = nc.dram_tensor("kc", [4, 128, 8, QC], BF16, kind="ExternalInput")
    vc_in = nc.dram_tensor("vc", [4, 128, 8, QC], BF16, kind="ExternalInput")
    ff1_in = nc.dram_tensor("ff1", [8, 128, 4, FFCP], BF16, kind="ExternalInput")
    ff2_in = nc.dram_tensor("ff2", [FFKT, 128, H], BF16, kind="ExternalInput")

    xnew_out = nc.dram_tensor("xnew_out", [128, KT], F32, kind="ExternalOutput")
    ff_out = nc.dram_tensor("ff_out", [H], F32, kind="ExternalOutput")

    with tile.TileContext(nc) as tc:
        with (
            tc.tile_pool(name="stream", bufs=11) as stream,
            tc.tile_pool(name="prodp", bufs=1) as prodp,
            tc.tile_pool(name="small", bufs=1) as small,
            tc.tile_pool(name="work", bufs=1) as work,
            tc.tile_pool(name="ps_row", bufs=4, space="PSUM") as ps_row,
            tc.tile_pool(name="ps_qb", bufs=1, space="PSUM") as ps_qb,
            tc.tile_pool(name="ps_small", bufs=2, space="PSUM") as ps_small,
            tc.tile_pool(name="dram", bufs=1, space="DRAM") as dram,
        ):
            # ---------------- constants + small loads ----------------
            ones_row = small.tile([1, 128], F32, tag="c0")
            ones_col = small.tile([128, 1], F32, tag="c1")
            nc.vector.memset(ones_row[:], 1.0)
            nc.vector.memset(ones_col[:], 1.0)
            ones_row_bf = small.tile([1, 128], BF16, tag="c3")
            nc.vector.memset(ones_row_bf[:], 1.0)
            eps_t = small.tile([1, 1], F32, tag="eps")
            nc.vector.memset(eps_t[:], EPS)
            shift_t = small.tile([128, 1], F32, tag="shift")
            nc.vector.memset(shift_t[:], EXP_SHIFT)
            warm = work.tile([1, 1], F32, tag="warm")

            x_cols = small.tile([128, KT], F32, tag="xc")
            an_cols = small.tile([128, KT], F32, tag="anc")
            fn_cols = small.tile([128, KT], F32, tag="fnc")
            rope_rows = [
                small.tile([1, QC], F32, tag=f"rope{r}", name=f"rope{r}")
                for r in range(4)
            ]
            eye32 = small.tile([32, 32], F32, tag="eye32")
            nc.gpsimd.dma_start(x_cols[:], x_in.ap())
            nc.gpsimd.dma_start(an_cols[:], an_in.ap())
            nc.gpsimd.dma_start(fn_cols[:], fn_in.ap())
            for r in range(4):
                nc.gpsimd.dma_start(rope_rows[r][:], rope_in.ap()[r : r + 1, :])
            nc.gpsimd.dma_start(eye32[:], eye32_in.ap())

            # preload the sqrt ACT table (first function used, by rmsnorm 1)
            nc.scalar.activation(warm[:], eps_t[:], AF.Sqrt)

            def rmsnorm(x_t, norm_t, out_t, nm):
                """out = x * norm * rsqrt(mean(x^2) + eps), [128, KT] cols."""
                scr = work.tile([128, KT], F32, tag=f"rms_scr{nm}")
                ssq = work.tile([128, 1], F32, tag=f"rms_ssq{nm}")
                nc.vector.scalar_tensor_tensor(
                    out=scr[:], in0=x_t[:], scalar=1.0, in1=x_t[:],
                    op0=ALU.mult, op1=ALU.mult, accum_out=ssq[:],
                )
                tot = ps_small.tile([1, 1], F32, tag="sm", name=f"rmst{nm}")
                nc.tensor.matmul(tot[:], ones_col[:], ssq[:], start=True, stop=True)
                rms = work.tile([1, 1], F32, tag=f"rms_rms{nm}")
                nc.scalar.activation(rms[:], tot[:], AF.Sqrt, bias=eps_t[:], scale=1.0 / H)
                rinv = work.tile([1, 1], F32, tag=f"rms_rinv{nm}")
                nc.vector.reciprocal(rinv[:], rms[:])
                rb_ps = ps_small.tile([128, 1], F32, tag="sm", name=f"rmsb{nm}")
                nc.tensor.matmul(rb_ps[:], ones_row[:], rinv[:], start=True, stop=True)
                rb_sb = work.tile([128, 1], F32, tag=f"rms_rb{nm}")
                nc.vector.tensor_copy(rb_sb[:], rb_ps[:])
                scl = work.tile([128, KT], F32, tag=f"rms_scl{nm}")
                nc.scalar.activation(scl[:], x_t[:], AF.Copy, scale=rb_sb[:])
                nc.vector.tensor_mul(out_t[:], scl[:], norm_t[:])

            # ---------------- rmsnorm 1 (h in bf16) ----------------
            h_bf = small.tile([128, KT], BF16, tag="hbf")
            rmsnorm(x_cols, an_cols, h_bf, "1")
            # prefetch the exp table (next function needed, for softmax)
            nc.scalar.activation(warm[:], eps_t[:], AF.Exp)

            # ---------------- stream DMAs (one HWDGE queue, usage order) ------
            wq_c = [None] * 4
            wk_c = [None] * 4
            wv_c = [None] * 4
            for g in range(4):
                wq_c[g] = stream.tile([128, 8, QC], BF16, tag="stream", name=f"wq{g}")
                nc.sync.dma_start(wq_c[g][:], wq_in.ap()[g])
            for g in range(4):
                wk_c[g] = stream.tile([128, 8, QC], BF16, tag="stream", name=f"wk{g}")
                nc.sync.dma_start(wk_c[g][:], wk_in.ap()[g])
            # K-cache chunks right after wk so scores can start early
            kch = [None] * n_kv_chunks
            for c in range(n_kv_chunks):
                kch[c] = stream.tile([128, 8, QC], BF16, tag="stream", name=f"kc{c}")
                s_hi = min(8, n_tiles - c * 8)
                full = (c * 8 + s_hi) * 128 <= n_s
                n_full_s = s_hi if full else s_hi - 1
                if n_full_s > 0:
                    nc.sync.dma_start(kch[c][:, 0:n_full_s, :], kc_in.ap()[c][:, 0:n_full_s, :])
                if not full:
                    nc.sync.dma_start(kch[c][0:rem, s_hi - 1, :], kc_in.ap()[c][0:rem, s_hi - 1, :])
            for g in range(4):
                wv_c[g] = stream.tile([128, 8, QC], BF16, tag="stream", name=f"wv{g}")
                nc.sync.dma_start(wv_c[g][:], wv_in.ap()[g])
            vch = [None] * n_kv_chunks
            for c in range(n_kv_chunks):
                vch[c] = stream.tile([128, 8, QC], BF16, tag="stream", name=f"vc{c}")
                s_hi = min(8, n_tiles - c * 8)
                full = (c * 8 + s_hi) * 128 <= n_s
                n_full_s = s_hi if full else s_hi - 1
                if n_full_s > 0:
                    nc.sync.dma_start(vch[c][:, 0:n_full_s, :], vc_in.ap()[c][:, 0:n_full_s, :])
                if not full:
                    nc.sync.dma_start(vch[c][0:rem, s_hi - 1, :], vc_in.ap()[c][0:rem, s_hi - 1, :])
            wo_c = [None] * 4
            for g in range(4):
                wo_c[g] = stream.tile([128, H], BF16, tag="stream", name=f"wo{g}")
                nc.sync.dma_start(wo_c[g][:], wo_in.ap()[g])
            ff1_c = [None] * 8
            for g in range(8):
                ff1_c[g] = stream.tile([128, 4, FFCP], BF16, tag="stream", name=f"f1{g}")
                nc.sync.dma_start(ff1_c[g][:], ff1_in.ap()[g])
            ff2_c = [None] * FFKT
            for g in range(FFKT):
                ff2_c[g] = stream.tile([128, H], BF16, tag="stream", name=f"f2{g}")
                nc.sync.dma_start(ff2_c[g][:], ff2_in.ap()[g])

            # ---------------- RoPE (f32 in, bf16 rows out) ----------------
            # rope_tbl rows: 0=cos, 1=sin, 2=cos*SCALE, 3=sin*SCALE
            def rope(src, cos_t, sin_t, out_row, nm):
                sv = src.rearrange("p (h t d) -> p h t d", h=HPC, t=2)
                rot = work.tile([1, HPC, 2, 64], F32, tag=f"rot_{nm}", name=f"rot{nm}")
                nc.scalar.activation(rot[:, :, 0, :], sv[:, :, 1, :], AF.Copy, scale=-1.0)
                nc.scalar.activation(rot[:, :, 1, :], sv[:, :, 0, :], AF.Copy, scale=1.0)
                t1 = work.tile([1, QC], F32, tag="t1", name=f"t1{nm}")
                t2 = work.tile([1, QC], F32, tag="t2", name=f"t2{nm}")
                nc.vector.tensor_mul(t1[:], src, cos_t)
                nc.vector.tensor_mul(t2[:], rot[:].rearrange("p h t d -> p (h t d)"), sin_t)
                nc.vector.tensor_add(out_row[:], t1[:], t2[:])

            q_row = small.tile([1, QC], BF16, tag="q_row")
            k_row = small.tile([1, QC], BF16, tag="k_row")
            v_row = small.tile([1, QC], BF16, tag="v_row")
            qb = small.tile([128, QC], BF16, tag="qb")

            # ---------------- q/k/v GEMV (one bank, rows 0/32/64) -------------
            q_ps_t = ps_row.tile([1, QC], F32, tag="row", name="qps")
            k_ps_t = ps_row.tile([1, QC], F32, tag="row", name="kps")
            v_ps_t = ps_row.tile([1, QC], F32, tag="row", name="vps")
            q_ps, k_ps, v_ps = q_ps_t[:], k_ps_t[:], v_ps_t[:]
            # separate loops in stream-arrival order (wq, wk, then wv after
            # kc) so the PE FIFO never blocks on a not-yet-streamed chunk
            for g in range(4):
                for j in range(8):
                    kt = g * 8 + j
                    st, sp = (kt == 0), (kt == KT - 1)
                    nc.tensor.matmul(q_ps, h_bf[:, kt : kt + 1], wq_c[g][:, j, :], start=st, stop=sp)
            # rope-q + qb broadcast live between the q and k GEMVs in the PE
            # FIFO so scores can start as soon as the K cache streams in
            def rope_q_and_broadcast():
                rope(q_ps, rope_rows[2][:], rope_rows[3][:], q_row, "q")
                qb_ps = ps_qb.tile([128, QC], F32, tag="qbps", name="qbps")
                nc.tensor.matmul(qb_ps[:], ones_row_bf[:], q_row[:], start=True, stop=True)
                nc.vector.tensor_copy(qb[:], qb_ps[:])

            rope_q_and_broadcast()
            for g in range(4):
                for j in range(8):
                    kt = g * 8 + j
                    st, sp = (kt == 0), (kt == KT - 1)
                    nc.tensor.matmul(k_ps, h_bf[:, kt : kt + 1], wk_c[g][:, j, :], start=st, stop=sp)
            for g in range(4):
                for j in range(8):
                    kt = g * 8 + j
                    st, sp = (kt == 0), (kt == KT - 1)
                    nc.tensor.matmul(v_ps, h_bf[:, kt : kt + 1], wv_c[g][:, j, :], start=st, stop=sp)

            # keep the PE warm between the v GEMV and o@V (the PE would
            # otherwise idle ~15us and the HAM clock gate would re-throttle)
            warm_mid = ps_qb.tile([1, 512], F32, tag="qbps", name="warmmid")
            for i in range(24):
                nc.tensor.matmul(
                    warm_mid[:],
                    h_bf[:, 0:1],
                    wv_c[3][:, i % 8, :],
                    start=(i == 0),
                    stop=(i == 23),
                )

            # rope-k runs on gpsimd so it never blocks the DVE score pipeline
            # behind the stream-gated k GEMV (ACT hops PSUM -> SBUF first)
            k_sb = work.tile([1, QC], F32, tag="k_sb")
            nc.scalar.copy(k_sb[:], k_ps)
            k_sv = k_sb[:].rearrange("p (h t d) -> p h t d", h=HPC, t=2)
            rot_k = work.tile([1, HPC, 2, 64], F32, tag="rot_k")
            nc.gpsimd.tensor_scalar_mul(rot_k[:, :, 0, :], k_sv[:, :, 1, :], -1.0)
            nc.gpsimd.tensor_copy(rot_k[:, :, 1, :], k_sv[:, :, 0, :])
            t1k = work.tile([1, QC], F32, tag="t1k")
            t2k = work.tile([1, QC], F32, tag="t2k")
            nc.gpsimd.tensor_mul(t1k[:], k_sb[:], rope_rows[0][:])
            nc.gpsimd.tensor_mul(
                t2k[:], rot_k[:].rearrange("p h t d -> p (h t d)"), rope_rows[1][:]
            )
            nc.gpsimd.tensor_add(k_row[:], t1k[:], t2k[:])
            nc.scalar.copy(v_row[:], v_ps)

            # insert current token's k into the streamed cache chunk (the v
            # insert happens after the score multiplies so it doesn't block
            # the gpsimd FIFO waiting for v_row)
            nc.gpsimd.dma_start(
                kch[pos_tile // 8][pos_row : pos_row + 1, pos_tile % 8, :], k_row[:]
            )

            # ---------------- attention, pipelined per KV chunk ---------------
            # scores_all[p, h, st] = q[h] . k[st*128+p, h] (pre-scaled via q)
            scores_all = small.tile([128, HPC, KT], F32, tag="scores")
            nc.vector.memset(scores_all[:], -1e30)
            p_stride = qb.ap[0][0]
            qb_bcast = bass.AP(
                tensor=qb.tensor,
                offset=qb.offset,
                ap=[[p_stride, 128], [0, 8], [1, QC]],
            )
            # exps_pad[p, st, 32h] = exp(score) for head h; other free slots
            # are zeroed so the [w, 97] lhsT loads garbage-free rows
            exps_pad = small.tile([128, KT, 97], BF16, tag="expad")
            nc.gpsimd.memset(exps_pad[:], 0.0)
            sums_c = [
                work.tile([128, HPC], F32, tag=f"sums{c}", name=f"sums{c}")
                for c in range(n_kv_chunks)
            ]
            # o accumulator: out[32h, h*HD:(h+1)*HD] holds head h
            o_ps = ps_row.tile([97, QC], F32, tag="row", name="ops")
            last_t = n_tiles - 1
            for c in range(n_kv_chunks):
                s_hi = min(8, n_tiles - c * 8)
                full = (c * 8 + s_hi) * 128 <= n_s and s_hi == 8
                if full:
                    prod = prodp.tile([128, 8, QC], BF16, tag="prod")
                    nc.vector.tensor_mul(prod[:], kch[c][:], qb_bcast)
                    nc.vector.reduce_sum(
                        scores_all[:, :, c * 8 : (c + 1) * 8],
                        prod[:].rearrange("p s (h d) -> p h s d", h=HPC),
                        axis=AX.X,
                    )
                else:
                    sp_stride = scores_all.ap[0][0]
                    for s in range(s_hi):
                        stt = c * 8 + s
                        w = 128 if (stt + 1) * 128 <= n_s else rem
                        scr = work.tile([128, 128], F32, tag="ttr_scr")
                        for hh in range(HPC):
                            acc_ap = bass.AP(
                                tensor=scores_all.tensor,
                                offset=scores_all.offset + hh * KT + stt,
                                ap=[[sp_stride, w], [1, 1]],
                            )
                            nc.vector.scalar_tensor_tensor(
                                out=scr[0:w, :],
                                in0=kch[c][0:w, s, hh * HD : (hh + 1) * HD],
                                scalar=1.0,
                                in1=qb[0:w, hh * HD : (hh + 1) * HD],
                                op0=ALU.mult,
                                op1=ALU.mult,
                                accum_out=acc_ap,
                            )
                # exp with constant shift: no cross-chunk barrier
                for hh in range(HPC):
                    nc.scalar.activation(
                        exps_pad[:, c * 8 : c * 8 + s_hi, 32 * hh],
                        scores_all[:, hh, c * 8 : c * 8 + s_hi],
                        AF.Exp,
                        bias=shift_t[:, 0:1],
                        scale=1.0,
                        accum_out=sums_c[c][:, hh : hh + 1],
                    )
                # insert the current token's v right before this chunk's o@V
                # (after the score TTs in the gpsimd FIFO, so it doesn't
                # stall them waiting for v_row)
                if c == pos_tile // 8:
                    nc.gpsimd.dma_start(
                        vch[c][pos_row : pos_row + 1, pos_tile % 8, :], v_row[:]
                    )
                # o += exps_chunk @ V_chunk: one 512-wide matmul per s-tile
                for s in range(s_hi):
                    stt = c * 8 + s
                    w = 128 if (stt + 1) * 128 <= n_s else rem
                    nc.tensor.matmul(
                        o_ps[:],
                        exps_pad[0:w, stt, :],
                        vch[c][0:w, s, :],
                        start=(stt == 0),
                        stop=(stt == last_t),
                    )

            # ---------------- softmax denominator + o finalize ----------------
            sums = work.tile([128, HPC], F32, tag="sums_t")
            if n_kv_chunks == 1:
                sums = sums_c[0]
            else:
                nc.vector.tensor_add(sums[:], sums_c[0][:], sums_c[1][:])
                for c in range(2, n_kv_chunks):
                    nc.vector.tensor_add(sums[:], sums[:], sums_c[c][:])
            tot4_ps = ps_small.tile([1, HPC], F32, tag="sm", name="tot4")
            nc.tensor.matmul(tot4_ps[:], ones_col[:], sums[:], start=True, stop=True)
            tot4 = work.tile([1, HPC], F32, tag="tot4_sb")
            nc.vector.tensor_copy(tot4[:], tot4_ps[:])
            rec4 = work.tile([1, HPC], F32, tag="rec4")
            nc.vector.reciprocal(rec4[:], tot4[:])
            # broadcast 1/sum to all partitions: rb4b[p, h] = rec4[h]
            rb4b_ps = ps_small.tile([128, HPC], F32, tag="sm", name="rb4b")
            nc.tensor.matmul(rb4b_ps[:], ones_row[:], rec4[:], start=True, stop=True)
            rb4b = work.tile([128, HPC], F32, tag="rb4b_sb")
            nc.vector.tensor_copy(rb4b[:], rb4b_ps[:])

            # scale each head's o by 1/sum while copying PSUM -> SBUF, then
            # transpose [1,128] rows (at base 0/32/64) into oT [128, 4]
            # scaled copies out of o_ps rows 0/32/64/96; head 3 lands at
            # (row 0, cols 128:256) because matmul lhsT can't sit at base 96
            o_sc = work.tile([65, 2 * HD], F32, tag="o_sc")
            sc_src = [(0, 0), (32, 32), (64, 64), (96, 0)]
            for hh in range(HPC):
                bi, bo = sc_src[hh]
                col = 0 if hh < 3 else HD
                nc.scalar.activation(
                    o_sc[bo : bo + 1, col : col + HD],
                    o_ps[bi : bi + 1, hh * HD : (hh + 1) * HD],
                    AF.Copy,
                    scale=rb4b[bi : bi + 1, hh : hh + 1],
                )
            oT_ps = ps_small.tile([128, HPC], F32, tag="sm", name="oTps")
            for hh in range(HPC):
                bo = sc_src[hh][1]
                col = 0 if hh < 3 else HD
                nc.tensor.transpose(
                    oT_ps[:, hh : hh + 1],
                    o_sc[bo : bo + 1, col : col + HD],
                    ones_col[bo : bo + 1, 0:1],
                )
            oT = work.tile([128, HPC], BF16, tag="oT_sb")
            nc.vector.tensor_copy(oT[:], oT_ps[:])

            # ---------------- attn partial row = o @ wo ----------------
            xattn = work.tile([1, H], BF16, tag="xattn")
            wo_ps_t = [
                ps_row.tile([97, 512], F32, tag="row", name=f"wops{i}")
                for i in range(3)
            ]
            for i in range(8):
                dst = wo_ps_t[i // 3][(i % 3) * 32 : (i % 3) * 32 + 1, :]
                for c in range(4):
                    nc.tensor.matmul(
                        dst,
                        oT[:, c : c + 1],
                        wo_c[c][:, i * 512 : (i + 1) * 512],
                        start=(c == 0),
                        stop=(c == 3),
                    )
                nc.scalar.copy(xattn[0:1, i * 512 : (i + 1) * 512], dst)

            # ---------------- AllReduce attn partial (bf16) -------------
            ar_in = dram.tile([H], BF16)
            ar_gath = dram.tile([CORES, H], BF16)
            nc.gpsimd.dma_start(ar_in[:].rearrange("(p n) -> p n", p=1), xattn[:])
            nc.gpsimd.collective_compute(
                "AllGather",
                ALU.bypass,
                replica_groups=[list(range(CORES))],
                ins=[ar_in[:].opt()],
                outs=[ar_gath[:].opt()],
            )
            # keep the PE busy (and the HAM clock gate open) while the
            # AllReduce runs: dummy accumulations over resident weights
            warm_ps = ps_qb.tile([1, 512], F32, tag="qbps", name="warmps")
            for i in range(N_WARM):
                nc.tensor.matmul(
                    warm_ps[:],
                    h_bf[:, 0:1],
                    wo_c[3][:, (i % 8) * 512 : (i % 8) * 512 + 512],
                    start=(i == 0),
                    stop=(i == N_WARM - 1),
                )

            # gathered [8 cores, 4096] -> SBUF [32 t, 8 c, 128 p] -> DVE
            # reduce over c -> rows [32, 128] -> PE transpose -> cols
            g_sb = work.tile([32, CORES, 128], BF16, tag="g_sb")
            nc.gpsimd.dma_start(
                g_sb[:], ar_gath[:].rearrange("c (t p) -> t c p", p=128)
            )
            ar_rows = work.tile([32, 128], F32, tag="ar_rows")
            g_view = bass.AP(
                tensor=g_sb.tensor,
                offset=g_sb.offset,
                ap=[list(g_sb.ap[0]), [1, 128], [128, CORES]],
            )
            nc.vector.reduce_sum(ar_rows[:], g_view, axis=AX.X)
            at_ps = ps_small.tile([128, 32], F32, tag="sm", name="atps")
            nc.tensor.transpose(at_ps[:], ar_rows[:], eye32[:])

            # ---------------- residual + rmsnorm 2 ----------------
            xnew = small.tile([128, KT], F32, tag="xnew")
            nc.vector.tensor_add(xnew[:], x_cols[:], at_ps[:])
            nc.gpsimd.dma_start(xnew_out.ap(), xnew[:])
            h2_bf = small.tile([128, KT], BF16, tag="h2bf")
            rmsnorm(xnew, fn_cols, h2_bf, "2")
            # prefetch the sigmoid table (next function, for silu)
            nc.scalar.activation(warm[:], eps_t[:], AF.Sigmoid)

            # ---------------- ff1 GEMV (one bank, rows 0/32/64) ---------------
            ff1_ps = ps_row.tile([65, 512], F32, tag="row", name="f1ps")
            f1_slots = [
                (ff1_ps[0:1, :], 0, 512),
                (ff1_ps[32:33, :], 512, 1024),
                (ff1_ps[64:65, 0:384], 1024, FFCP),
            ]
            for g in range(8):
                for j in range(4):
                    kt = g * 4 + j
                    st, sp = (kt == 0), (kt == KT - 1)
                    lhs = h2_bf[:, kt : kt + 1]
                    for slot, lo, hi in f1_slots:
                        nc.tensor.matmul(
                            slot, lhs, ff1_c[g][:, j, lo:hi], start=st, stop=sp
                        )

            # ---------------- silu on rows 0/32/64, then PE transposes --------
            sg = work.tile([65, 512], F32, tag="sg")
            nc.scalar.activation(sg[:], ff1_ps[:], AF.Sigmoid)
            silu65 = work.tile([65, 512], F32, tag="silu")
            nc.vector.tensor_mul(silu65[:], sg[:], ff1_ps[:])
            siluT_ps = ps_small.tile([128, FFKT], F32, tag="sm", name="siluTps")
            for t in range(FFKT):
                r, off = (t * 128) // 512, (t * 128) % 512
                b = [0, 32, 64][r]
                nc.tensor.transpose(
                    siluT_ps[:, t : t + 1],
                    silu65[b : b + 1, off : off + 128],
                    ones_col[b : b + 1, 0:1],
                )
            silu_cols = work.tile([128, FFKT], BF16, tag="silu_cols")
            nc.vector.tensor_copy(silu_cols[:], siluT_ps[:])

            # ---------------- ff2 GEMV (8 strips in 3 banks, single pass) -----
            ffrow = work.tile([1, H], F32, tag="ffrow")
            f2_t = [
                ps_row.tile([97, 512], F32, tag="row", name=f"f2ps{i}")
                for i in range(3)
            ]
            f2_slots = [
                f2_t[i // 3][(i % 3) * 32 : (i % 3) * 32 + 1, :] for i in range(8)
            ]
            for kt in range(FFKT):
                lhs = silu_cols[:, kt : kt + 1]
                for i in range(8):
                    nc.tensor.matmul(
                        f2_slots[i],
                        lhs,
                        ff2_c[kt][:, i * 512 : (i + 1) * 512],
                        start=(kt == 0),
                        stop=(kt == FFKT - 1),
                    )
            for i in range(8):
                nc.scalar.copy(ffrow[0:1, i * 512 : (i + 1) * 512], f2_slots[i])
            nc.gpsimd.dma_start(ff_out.ap().rearrange("(p n) -> p n", p=1), ffrow[:])

    nc.compile()
    _BUILD_CACHE[pos] = nc
    return nc


def _pack_k8(w):
    """[4096, N] -> [4, 128, 8, N] tile-major (chunk g, partition p, j, n)."""
    n = w.shape[1]
    return np.ascontiguousarray(
        w.reshape(4, 8, 128, n).transpose(0, 2, 1, 3)
    )


def _shard(inputs, pos):
    f32 = np.float32
    x = np.asarray(inputs["x"], f32)
    an = np.asarray(inputs["attn_norm"], f32)
    fn = np.asarray(inputs["ffn_norm"], f32)
    cos_r = np.asarray(inputs["cos_cache"], f32)[pos]
    sin_r = np.asarray(inputs["sin_cache"], f32)[pos]
    wq = np.asarray(inputs["w_q"], f32)
    wk = np.asarray(inputs["w_k"], f32)
    wv = np.asarray(inputs["w_v"], f32)
    wo = np.asarray(inputs["w_o"], f32)
    kc = np.asarray(inputs["k_cache"], f32)
    vc = np.asarray(inputs["v_cache"], f32)
    ff1 = np.asarray(inputs["w_ff1"], f32)
    ff2 = np.asarray(inputs["w_ff2"], f32)

    def cols(v):
        return np.ascontiguousarray(v.reshape(KT, 128).T)

    x_c, an_c, fn_c = cols(x), cols(an), cols(fn)
    rope_tbl = np.stack(
        [
            np.tile(cos_r, HPC),
            np.tile(sin_r, HPC),
            np.tile(cos_r, HPC) * SCALE,
            np.tile(sin_r, HPC) * SCALE,
        ]
    ).astype(f32)
    eye32 = np.eye(32, dtype=f32)

    in_maps = []
    for c in range(CORES):
        qlo, qhi = c * QC, (c + 1) * QC
        flo, fhi = c * FFC, (c + 1) * FFC
        hlo, hhi = c * HPC, (c + 1) * HPC
        kc_c = kc[:, hlo:hhi, :].reshape(H, QC).astype(NP_BF16)
        vc_c = vc[:, hlo:hhi, :].reshape(H, QC).astype(NP_BF16)
        ff1_c = np.zeros((H, FFCP), dtype=NP_BF16)
        ff1_c[:, :FFC] = ff1[:, flo:fhi].astype(NP_BF16)
        ff2_c = np.zeros((FFKT * 128, H), dtype=NP_BF16)
        ff2_c[:FFC, :] = ff2[flo:fhi, :].astype(NP_BF16)
        in_maps.append(
            {
                "x_cols": x_c,
                "an_cols": an_c,
                "fn_cols": fn_c,
                "rope_tbl": rope_tbl,
                "eye32": eye32,
                "wq": _pack_k8(wq[:, qlo:qhi].astype(NP_BF16)),
                "wk": _pack_k8(wk[:, qlo:qhi].astype(NP_BF16)),
                "wv": _pack_k8(wv[:, qlo:qhi].astype(NP_BF16)),
                "wo": np.ascontiguousarray(
                    wo[qlo:qhi, :].astype(NP_BF16).reshape(4, 128, H)
                ),
                "kc": _pack_k8(kc_c),
                "vc": _pack_k8(vc_c),
                "ff1": np.ascontiguousarray(
                    ff1_c.reshape(8, 4, 128, FFCP).transpose(0, 2, 1, 3)
                ),
                "ff2": np.ascontiguousarray(ff2_c.reshape(FFKT, 128, H)),
            }
        )
    return in_maps


def _assemble(results):
    xnew_cols = results[0]["xnew_out"]  # [128, 32], element (p,t) = vec[t*128+p]
    xnew = np.ascontiguousarray(xnew_cols.T).reshape(-1)
    ff = np.sum(
        np.stack([results[c]["ff_out"] for c in range(CORES)]), axis=0,
        dtype=np.float32,
    )
    return (xnew + ff).astype(np.float32)


def run(inputs, trace=False):
    pos = int(inputs["pos"])
    nc = _build(pos)
    in_maps = _shard(inputs, pos)
    res = run_bass_kernel_spmd(nc, in_maps, core_ids=list(range(CORES)), trace=trace)
    return _assemble(res.results), res


def kernel(**inputs) -> np.ndarray:
    out, _ = run(inputs, trace=False)
    return out


# revision 27
# speedup vs baseline: 1.0949x; 1.0949x over previous
"""Llama block (single-token decode) on 8 TRN2 NeuronCores, tensor-parallel.

Sharding (per core c of 8):
  - heads 4c..4c+3: w_q/w_k/w_v column shards [4096, 512], KV cache [4096, 4, 128]
  - w_o row shard [512, 4096] -> partial attn output, AllReduce'd on device
  - w_ff1 column shard [4096, 1376->1408pad], w_ff2 row shard [1376->1408pad, 4096]
  - per-core FFN partials summed on host (row-sharded output unshard)

Perf design vs the f32 baseline:
  - All large tensors cast to bf16 ON HOST and repacked tile-major so every
    streaming DMA is a contiguous ~1 MB transfer (descriptor-efficient).
  - One HWDGE queue (nc.sync) carries the 43 stream chunks in exact
    consumption order: wq, wk, kc, wv, vc, wo, ff1, ff2.  All small or
    dependency-carrying DMAs ride SWDGE (nc.gpsimd) so the stream never
    blocks behind the AllReduce.
  - Softmax uses a constant shift (exp(s - B), B=20) instead of the true
    max: mathematically identical after normalization, and it removes the
    all-chunk barrier - each KV chunk flows scores -> exp -> o@V pipelined.
  - Scores are batched DVE ops (one mult + one segmented reduce per chunk).
  - PSUM accumulation groups are packed three-per-bank at base partitions
    0/32/64 (qkv in one bank, ff2's 8 strips in 3 banks, single pass).
  - PE warm-up spinner matmuls bridge the AllReduce gap so the HAM clock
    gate stays open for the ff1/ff2 GEMVs.
  - ACT table loads (sqrt/exp/sigmoid) are prefetched by dummy ops placed
    right after the previous function's last real use.
  - AllReduce payload is bf16 (8 KB) and overlaps the ff weight stream.

On-chip layout convention: a length-4096 vector is held as [128, 32] "cols"
(element (p, t) = vec[t*128 + p]) so vector tiles feed matmul lhsT directly.
"""

import math
import sys

sys.path.insert(0, "/opt/trn_rl_repo")

import numpy as np
import ml_dtypes

import concourse.bass as bass
import concourse.tile as tile
from concourse import bacc, mybir
from concourse.bass_utils import run_bass_kernel_spmd
from concourse import bass_isa

F32 = mybir.dt.float32
BF16 = mybir.dt.bfloat16
AF = mybir.ActivationFunctionType
ALU = mybir.AluOpType
AX = mybir.AxisListType

H = 4096
NH = 32
HD = 128
INTERM = 11008
EPS = 1e-6
CORES = 8
HPC = NH // CORES  # 4 heads per core
QC = HPC * HD  # 512 qkv cols per core
FFC = INTERM // CORES  # 1376 ff cols per core
FFCP = 1408  # padded to 11 * 128
FFKT = FFCP // 128  # 11 contraction tiles for ff2
KT = H // 128  # 32 contraction tiles
SCALE = 1.0 / math.sqrt(HD)
EXP_SHIFT = -20.0  # constant softmax shift; |scores| << 20 for this model
NP_BF16 = ml_dtypes.bfloat16
N_WARM = 96  # PE warm-up spinner matmuls bridging the AllGather

_BUILD_CACHE = {}


def _build(pos: int):
    if pos in _BUILD_CACHE:
        return _BUILD_CACHE[pos]

    n_s = pos + 1
    n_tiles = (n_s + 127) // 128  # s-tiles to attend over
    rem = n_s - (n_tiles - 1) * 128  # rows in last s-tile (1..128)
    pos_tile = pos // 128
    pos_row = pos % 128
    n_kv_chunks = (n_tiles + 7) // 8

    nc = bacc.Bacc("TRN2", target_bir_lowering=False, debug=False, num_devices=CORES)

    x_in = nc.dram_tensor("x_cols", [128, KT], F32, kind="ExternalInput")
    an_in = nc.dram_tensor("an_cols", [128, KT], F32, kind="ExternalInput")
    fn_in = nc.dram_tensor("fn_cols", [128, KT], F32, kind="ExternalInput")
    rope_in = nc.dram_tensor("rope_tbl", [4, QC], F32, kind="ExternalInput")
    eye32_in = nc.dram_tensor("eye32", [32, 32], F32, kind="ExternalInput")
    wq_in = nc.dram_tensor("wq", [4, 128, 8, QC], BF16, kind="ExternalInput")
    wk_in = nc.dram_tensor("wk", [4, 128, 8, QC], BF16, kind="ExternalInput")
    wv_in = nc.dram_tensor("wv", [4, 128, 8, QC], BF16, kind="ExternalInput")
    wo_in = nc.dram_tensor("wo", [4, 128, H], BF16, kind="ExternalInput")
    kc_in = nc.dram_tensor("kc", [4, 128, 8, QC], BF16, kind="ExternalInput")
    vc_in = nc.dram_tensor("vc", [4, 128, 8, QC], BF16, kind="ExternalInput")
    ff1_in = nc.dram_tensor("ff1", [8, 128, 4, FFCP], BF16, kind="ExternalInput")
    ff2_in = nc.dram_tensor("ff2", [FFKT, 128, H], BF16, kind="ExternalInput")

    xnew_out = nc.dram_tensor("xnew_out", [128, KT], F32, kind="ExternalOutput")
    ff_out = nc.dram_tensor("ff_out", [H], F32, kind="ExternalOutput")

    with tile.TileContext(nc) as tc:
        with (
            tc.tile_pool(name="stream", bufs=11) as stream,
            tc.tile_pool(name="prodp", bufs=1) as prodp,
            tc.tile_pool(name="small", bufs=1) as small,
            tc.tile_pool(name="work", bufs=1) as work,
            tc.tile_pool(name="ps_row", bufs=4, space="PSUM") as ps_row,
            tc.tile_pool(name="ps_qb", bufs=1, space="PSUM") as ps_qb,
            tc.tile_pool(name="ps_small", bufs=2, space="PSUM") as ps_small,
            tc.tile_pool(name="dram", bufs=1, space="DRAM") as dram,
        ):
            # ---------------- constants + small loads ----------------
            ones_row = small.tile([1, 128], F32, tag="c0")
            ones_col = small.tile([128, 1], F32, tag="c1")
            nc.vector.memset(ones_row[:], 1.0)
            nc.vector.memset(ones_col[:], 1.0)
            ones_row_bf = small.tile([1, 128], BF16, tag="c3")
            nc.vector.memset(ones_row_bf[:], 1.0)
            eps_t = small.tile([1, 1], F32, tag="eps")
            nc.vector.memset(eps_t[:], EPS)
            shift_t = small.tile([128, 1], F32, tag="shift")
            nc.vector.memset(shift_t[:], EXP_SHIFT)
            warm = work.tile([1, 1], F32, tag="warm")

            x_cols = small.tile([128, KT], F32, tag="xc")
            an_cols = small.tile([128, KT], F32, tag="anc")
            fn_cols = small.tile([128, KT], F32, tag="fnc")
            rope_rows = [
                small.tile([1, QC], F32, tag=f"rope{r}", name=f"rope{r}")
                for r in range(4)
            ]
            eye32 = small.tile([32, 32], F32, tag="eye32")
            nc.gpsimd.dma_start(x_cols[:], x_in.ap())
            nc.gpsimd.dma_start(an_cols[:], an_in.ap())
            nc.gpsimd.dma_start(fn_cols[:], fn_in.ap())
            for r in range(4):
                nc.gpsimd.dma_start(rope_rows[r][:], rope_in.ap()[r : r + 1, :])
            nc.gpsimd.dma_start(eye32[:], eye32_in.ap())

            # preload the sqrt ACT table (first function used, by rmsnorm 1)
            nc.scalar.activation(warm[:], eps_t[:], AF.Sqrt)

            def rmsnorm(x_t, norm_t, out_t, nm):
                """out = x * norm * rsqrt(mean(x^2) + eps), [128, KT] cols."""
                scr = work.tile([128, KT], F32, tag=f"rms_scr{nm}")
                ssq = work.tile([128, 1], F32, tag=f"rms_ssq{nm}")
                nc.vector.scalar_tensor_tensor(
                    out=scr[:], in0=x_t[:], scalar=1.0, in1=x_t[:],
                    op0=ALU.mult, op1=ALU.mult, accum_out=ssq[:],
                )
                tot = ps_small.tile([1, 1], F32, tag="sm", name=f"rmst{nm}")
                nc.tensor.matmul(tot[:], ones_col[:], ssq[:], start=True, stop=True)
                rms = work.tile([1, 1], F32, tag=f"rms_rms{nm}")
                nc.scalar.activation(rms[:], tot[:], AF.Sqrt, bias=eps_t[:], scale=1.0 / H)
                rinv = work.tile([1, 1], F32, tag=f"rms_rinv{nm}")
                nc.vector.reciprocal(rinv[:], rms[:])
                rb_ps = ps_small.tile([128, 1], F32, tag="sm", name=f"rmsb{nm}")
                nc.tensor.matmul(rb_ps[:], ones_row[:], rinv[:], start=True, stop=True)
                rb_sb = work.tile([128, 1], F32, tag=f"rms_rb{nm}")
                nc.vector.tensor_copy(rb_sb[:], rb_ps[:])
                scl = work.tile([128, KT], F32, tag=f"rms_scl{nm}")
                nc.scalar.activation(scl[:], x_t[:], AF.Copy, scale=rb_sb[:])
                nc.vector.tensor_mul(out_t[:], scl[:], norm_t[:])

            # ---------------- rmsnorm 1 (h in bf16) ----------------
            h_bf = small.tile([128, KT], BF16, tag="hbf")
            rmsnorm(x_cols, an_cols, h_bf, "1")
            # prefetch the exp table (next function needed, for softmax)
            nc.scalar.activation(warm[:], eps_t[:], AF.Exp)

            # ---------------- stream DMAs (one HWDGE queue, usage order) ------
            wq_c = [None] * 4
            wk_c = [None] * 4
            wv_c = [None] * 4
            for g in range(4):
                wq_c[g] = stream.tile([128, 8, QC], BF16, tag="stream", name=f"wq{g}")
                nc.sync.dma_start(wq_c[g][:], wq_in.ap()[g])
            for g in range(4):
                wk_c[g] = stream.tile([128, 8, QC], BF16, tag="stream", name=f"wk{g}")
                nc.sync.dma_start(wk_c[g][:], wk_in.ap()[g])
            # K-cache chunks right after wk so scores can start early
            kch = [None] * n_kv_chunks
            for c in range(n_kv_chunks):
                kch[c] = stream.tile([128, 8, QC], BF16, tag="stream", name=f"kc{c}")
                s_hi = min(8, n_tiles - c * 8)
                full = (c * 8 + s_hi) * 128 <= n_s
                n_full_s = s_hi if full else s_hi - 1
                if n_full_s > 0:
                    nc.sync.dma_start(kch[c][:, 0:n_full_s, :], kc_in.ap()[c][:, 0:n_full_s, :])
                if not full:
                    nc.sync.dma_start(kch[c][0:rem, s_hi - 1, :], kc_in.ap()[c][0:rem, s_hi - 1, :])
            for g in range(4):
                wv_c[g] = stream.tile([128, 8, QC], BF16, tag="stream", name=f"wv{g}")
                nc.sync.dma_start(wv_c[g][:], wv_in.ap()[g])
            vch = [None] * n_kv_chunks
            for c in range(n_kv_chunks):
                vch[c] = stream.tile([128, 8, QC], BF16, tag="stream", name=f"vc{c}")
                s_hi = min(8, n_tiles - c * 8)
                full = (c * 8 + s_hi) * 128 <= n_s
                n_full_s = s_hi if full else s_hi - 1
                if n_full_s > 0:
                    nc.sync.dma_start(vch[c][:, 0:n_full_s, :], vc_in.ap()[c][:, 0:n_full_s, :])
                if not full:
                    nc.sync.dma_start(vch[c][0:rem, s_hi - 1, :], vc_in.ap()[c][0:rem, s_hi - 1, :])
            wo_c = [None] * 4
            for g in range(4):
                wo_c[g] = stream.tile([128, H], BF16, tag="stream", name=f"wo{g}")
                nc.sync.dma_start(wo_c[g][:], wo_in.ap()[g])
            ff1_c = [None] * 8
            for g in range(8):
                ff1_c[g] = stream.tile([128, 4, FFCP], BF16, tag="stream", name=f"f1{g}")
                nc.sync.dma_start(ff1_c[g][:], ff1_in.ap()[g])
            ff2_c = [None] * FFKT
            for g in range(FFKT):
                ff2_c[g] = stream.tile([128, H], BF16, tag="stream", name=f"f2{g}")
                nc.sync.dma_start(ff2_c[g][:], ff2_in.ap()[g])

            # ---------------- RoPE (f32 in, bf16 rows out) ----------------
            # rope_tbl rows: 0=cos, 1=sin, 2=cos*SCALE, 3=sin*SCALE
            def rope(src, cos_t, sin_t, out_row, nm):
                sv = src.rearrange("p (h t d) -> p h t d", h=HPC, t=2)
                rot = work.tile([1, HPC, 2, 64], F32, tag=f"rot_{nm}", name=f"rot{nm}")
                nc.scalar.activation(rot[:, :, 0, :], sv[:, :, 1, :], AF.Copy, scale=-1.0)
                nc.scalar.activation(rot[:, :, 1, :], sv[:, :, 0, :], AF.Copy, scale=1.0)
                t1 = work.tile([1, QC], F32, tag="t1", name=f"t1{nm}")
                t2 = work.tile([1, QC], F32, tag="t2", name=f"t2{nm}")
                nc.vector.tensor_mul(t1[:], src, cos_t)
                nc.vector.tensor_mul(t2[:], rot[:].rearrange("p h t d -> p (h t d)"), sin_t)
                nc.vector.tensor_add(out_row[:], t1[:], t2[:])

            q_row = small.tile([1, QC], BF16, tag="q_row")
            k_row = small.tile([1, QC], BF16, tag="k_row")
            v_row = small.tile([1, QC], BF16, tag="v_row")
            qb = small.tile([128, QC], BF16, tag="qb")

            # ---------------- q/k/v GEMV (one bank, rows 0/32/64) -------------
            q_ps_t = ps_row.tile([1, QC], F32, tag="row", name="qps")
            k_ps_t = ps_row.tile([1, QC], F32, tag="row", name="kps")
            v_ps_t = ps_row.tile([1, QC], F32, tag="row", name="vps")
            q_ps, k_ps, v_ps = q_ps_t[:], k_ps_t[:], v_ps_t[:]
            # separate loops in stream-arrival order (wq, wk, then wv after
            # kc) so the PE FIFO never blocks on a not-yet-streamed chunk
            for g in range(4):
                for j in range(8):
                    kt = g * 8 + j
                    st, sp = (kt == 0), (kt == KT - 1)
                    nc.tensor.matmul(q_ps, h_bf[:, kt : kt + 1], wq_c[g][:, j, :], start=st, stop=sp)
            # rope-q + qb broadcast live between the q and k GEMVs in the PE
            # FIFO so scores can start as soon as the K cache streams in
            def rope_q_and_broadcast():
                rope(q_ps, rope_rows[2][:], rope_rows[3][:], q_row, "q")
                qb_ps = ps_qb.tile([128, QC], F32, tag="qbps", name="qbps")
                nc.tensor.matmul(qb_ps[:], ones_row_bf[:], q_row[:], start=True, stop=True)
                nc.vector.tensor_copy(qb[:], qb_ps[:])

            rope_q_and_broadcast()
            for g in range(4):
                for j in range(8):
                    kt = g * 8 + j
                    st, sp = (kt == 0), (kt == KT - 1)
                    nc.tensor.matmul(k_ps, h_bf[:, kt : kt + 1], wk_c[g][:, j, :], start=st, stop=sp)
            for g in range(4):
                for j in range(8):
                    kt = g * 8 + j
                    st, sp = (kt == 0), (kt == KT - 1)
                    nc.tensor.matmul(v_ps, h_bf[:, kt : kt + 1], wv_c[g][:, j, :], start=st, stop=sp)

            # keep the PE warm between the v GEMV and o@V (the PE would
            # otherwise idle ~15us and the HAM clock gate would re-throttle)
            warm_mid = ps_qb.tile([1, 512], F32, tag="qbps", name="warmmid")
            for i in range(24):
                nc.tensor.matmul(
                    warm_mid[:],
                    h_bf[:, 0:1],
                    wv_c[3][:, i % 8, :],
                    start=(i == 0),
                    stop=(i == 23),
                )

            # rope-k runs on gpsimd so it never blocks the DVE score pipeline
            # behind the stream-gated k GEMV (ACT hops PSUM -> SBUF first)
            k_sb = work.tile([1, QC], F32, tag="k_sb")
            nc.scalar.copy(k_sb[:], k_ps)
            k_sv = k_sb[:].rearrange("p (h t d) -> p h t d", h=HPC, t=2)
            rot_k = work.tile([1, HPC, 2, 64], F32, tag="rot_k")
            nc.gpsimd.tensor_scalar_mul(rot_k[:, :, 0, :], k_sv[:, :, 1, :], -1.0)
            nc.gpsimd.tensor_copy(rot_k[:, :, 1, :], k_sv[:, :, 0, :])
            t1k = work.tile([1, QC], F32, tag="t1k")
            t2k = work.tile([1, QC], F32, tag="t2k")
            nc.gpsimd.tensor_mul(t1k[:], k_sb[:], rope_rows[0][:])
            nc.gpsimd.tensor_mul(
                t2k[:], rot_k[:].rearrange("p h t d -> p (h t d)"), rope_rows[1][:]
            )
            nc.gpsimd.tensor_add(k_row[:], t1k[:], t2k[:])
            nc.scalar.copy(v_row[:], v_ps)

            # insert current token's k into the streamed cache chunk (the v
            # insert happens after the score multiplies so it doesn't block
            # the gpsimd FIFO waiting for v_row)
            nc.gpsimd.dma_start(
                kch[pos_tile // 8][pos_row : pos_row + 1, pos_tile % 8, :], k_row[:]
            )

            # ---------------- attention, pipelined per KV chunk ---------------
            # scores_all[p, h, st] = q[h] . k[st*128+p, h] (pre-scaled via q)
            scores_all = small.tile([128, HPC, KT], F32, tag="scores")
            nc.gpsimd.memset(scores_all[:], -1e30)
            p_stride = qb.ap[0][0]
            qb_bcast = bass.AP(
                tensor=qb.tensor,
                offset=qb.offset,
                ap=[[p_stride, 128], [0, 8], [1, QC]],
            )
            # exps_pad[p, st, 32h] = exp(score) for head h; other free slots
            # are zeroed so the [w, 97] lhsT loads garbage-free rows
            exps_pad = small.tile([128, KT, 97], BF16, tag="expad")
            nc.gpsimd.memset(exps_pad[:], 0.0)
            sums_c = [
                work.tile([128, HPC], F32, tag=f"sums{c}", name=f"sums{c}")
                for c in range(n_kv_chunks)
            ]
            # o accumulator: out[32h, h*HD:(h+1)*HD] holds head h
            o_ps = ps_row.tile([97, QC], F32, tag="row", name="ops")
            last_t = n_tiles - 1
            for c in range(n_kv_chunks):
                s_hi = min(8, n_tiles - c * 8)
                full = (c * 8 + s_hi) * 128 <= n_s and s_hi == 8
                if full:
                    prod = prodp.tile([128, 8, QC], BF16, tag="prod")
                    nc.vector.tensor_mul(prod[:], kch[c][:], qb_bcast)
                    nc.vector.reduce_sum(
                        scores_all[:, :, c * 8 : (c + 1) * 8],
                        prod[:].rearrange("p s (h d) -> p h s d", h=HPC),
                        axis=AX.X,
                    )
                else:
                    sp_stride = scores_all.ap[0][0]
                    for s in range(s_hi):
                        stt = c * 8 + s
                        w = 128 if (stt + 1) * 128 <= n_s else rem
                        scr = work.tile([128, 128], F32, tag="ttr_scr")
                        for hh in range(HPC):
                            acc_ap = bass.AP(
                                tensor=scores_all.tensor,
                                offset=scores_all.offset + hh * KT + stt,
                                ap=[[sp_stride, w], [1, 1]],
                            )
                            nc.vector.scalar_tensor_tensor(
                                out=scr[0:w, :],
                                in0=kch[c][0:w, s, hh * HD : (hh + 1) * HD],
                                scalar=1.0,
                                in1=qb[0:w, hh * HD : (hh + 1) * HD],
                                op0=ALU.mult,
                                op1=ALU.mult,
                                accum_out=acc_ap,
                            )
                # exp with constant shift: no cross-chunk barrier
                for hh in range(HPC):
                    nc.scalar.activation(
                        exps_pad[:, c * 8 : c * 8 + s_hi, 32 * hh],
                        scores_all[:, hh, c * 8 : c * 8 + s_hi],
                        AF.Exp,
                        bias=shift_t[:, 0:1],
                        scale=1.0,
                        accum_out=sums_c[c][:, hh : hh + 1],
                    )
                # insert the current token's v right before this chunk's o@V
                # (after the score TTs in the gpsimd FIFO, so it doesn't
                # stall them waiting for v_row)
                if c == pos_tile // 8:
                    nc.gpsimd.dma_start(
                        vch[c][pos_row : pos_row + 1, pos_tile % 8, :], v_row[:]
                    )
                # o += exps_chunk @ V_chunk: one 512-wide matmul per s-tile
                for s in range(s_hi):
                    stt = c * 8 + s
                    w = 128 if (stt + 1) * 128 <= n_s else rem
                    nc.tensor.matmul(
                        o_ps[:],
                        exps_pad[0:w, stt, :],
                        vch[c][0:w, s, :],
                        start=(stt == 0),
                        stop=(stt == last_t),
                    )

            # ---------------- softmax denominator + o finalize ----------------
            sums = work.tile([128, HPC], F32, tag="sums_t")
            if n_kv_chunks == 1:
                sums = sums_c[0]
            else:
                nc.vector.tensor_add(sums[:], sums_c[0][:], sums_c[1][:])
                for c in range(2, n_kv_chunks):
                    nc.vector.tensor_add(sums[:], sums[:], sums_c[c][:])
            tot4_ps = ps_small.tile([1, HPC], F32, tag="sm", name="tot4")
            nc.tensor.matmul(tot4_ps[:], ones_col[:], sums[:], start=True, stop=True)
            tot4 = work.tile([1, HPC], F32, tag="tot4_sb")
            nc.vector.tensor_copy(tot4[:], tot4_ps[:])
            rec4 = work.tile([1, HPC], F32, tag="rec4")
            nc.vector.reciprocal(rec4[:], tot4[:])
            # broadcast 1/sum to all partitions: rb4b[p, h] = rec4[h]
            rb4b_ps = ps_small.tile([128, HPC], F32, tag="sm", name="rb4b")
            nc.tensor.matmul(rb4b_ps[:], ones_row[:], rec4[:], start=True, stop=True)
            rb4b = work.tile([128, HPC], F32, tag="rb4b_sb")
            nc.vector.tensor_copy(rb4b[:], rb4b_ps[:])

            # scale each head's o by 1/sum while copying PSUM -> SBUF, then
            # transpose [1,128] rows (at base 0/32/64) into oT [128, 4]
            # scaled copies out of o_ps rows 0/32/64/96; head 3 lands at
            # (row 0, cols 128:256) because matmul lhsT can't sit at base 96
            o_sc = work.tile([65, 2 * HD], F32, tag="o_sc")
            sc_src = [(0, 0), (32, 32), (64, 64), (96, 0)]
            for hh in range(HPC):
                bi, bo = sc_src[hh]
                col = 0 if hh < 3 else HD
                nc.scalar.activation(
                    o_sc[bo : bo + 1, col : col + HD],
                    o_ps[bi : bi + 1, hh * HD : (hh + 1) * HD],
                    AF.Copy,
                    scale=rb4b[bi : bi + 1, hh : hh + 1],
                )
            oT_ps = ps_small.tile([128, HPC], F32, tag="sm", name="oTps")
            for hh in range(HPC):
                bo = sc_src[hh][1]
                col = 0 if hh < 3 else HD
                nc.tensor.transpose(
                    oT_ps[:, hh : hh + 1],
                    o_sc[bo : bo + 1, col : col + HD],
                    ones_col[bo : bo + 1, 0:1],
                )
            oT = work.tile([128, HPC], BF16, tag="oT_sb")
            nc.vector.tensor_copy(oT[:], oT_ps[:])

            # ---------------- attn partial row = o @ wo ----------------
            wo_ps_t = [
                ps_row.tile([97, 512], F32, tag="row", name=f"wops{i}")
                for i in range(3)
            ]
            for i in range(8):
                dst = wo_ps_t[i // 3][(i % 3) * 32 : (i % 3) * 32 + 1, :]
                for c in range(4):
                    nc.tensor.matmul(
                        dst,
                        oT[:, c : c + 1],
                        wo_c[c][:, i * 512 : (i + 1) * 512],
                        start=(c == 0),
                        stop=(c == 3),
                    )
            xattn = work.tile([1, H], BF16, tag="xattn")
            for i in range(8):
                nc.scalar.copy(
                    xattn[0:1, i * 512 : (i + 1) * 512],
                    wo_ps_t[i // 3][(i % 3) * 32 : (i % 3) * 32 + 1, :],
                )

            # ---------------- AllReduce attn partial (bf16) -------------
            ar_in = dram.tile([H], BF16)
            ar_gath = dram.tile([CORES, H], BF16)
            nc.gpsimd.dma_start(ar_in[:].rearrange("(p n) -> p n", p=1), xattn[:])
            nc.gpsimd.collective_compute(
                "AllGather",
                ALU.bypass,
                replica_groups=[list(range(CORES))],
                ins=[ar_in[:].opt()],
                outs=[ar_gath[:].opt()],
            )
            # keep the PE busy (and the HAM clock gate open) while the
            # AllReduce runs: dummy accumulations over resident weights
            warm_ps = ps_qb.tile([1, 512], F32, tag="qbps", name="warmps")
            for i in range(N_WARM):
                nc.tensor.matmul(
                    warm_ps[:],
                    h_bf[:, 0:1],
                    wo_c[3][:, (i % 8) * 512 : (i % 8) * 512 + 512],
                    start=(i == 0),
                    stop=(i == N_WARM - 1),
                )

            # gathered [8 cores, 4096] -> SBUF [32 t, 8 c, 128 p] -> DVE
            # reduce over c -> rows [32, 128] -> PE transpose -> cols
            g_sb = work.tile([32, CORES, 128], BF16, tag="g_sb")
            nc.gpsimd.dma_start(
                g_sb[:], ar_gath[:].rearrange("c (t p) -> t c p", p=128)
            )
            ar_rows = work.tile([32, 128], F32, tag="ar_rows")
            g_view = bass.AP(
                tensor=g_sb.tensor,
                offset=g_sb.offset,
                ap=[list(g_sb.ap[0]), [1, 128], [128, CORES]],
            )
            nc.vector.reduce_sum(ar_rows[:], g_view, axis=AX.X)
            at_ps = ps_small.tile([128, 32], F32, tag="sm", name="atps")
            nc.tensor.transpose(at_ps[:], ar_rows[:], eye32[:])

            # ---------------- residual + rmsnorm 2 ----------------
            xnew = small.tile([128, KT], F32, tag="xnew")
            nc.vector.tensor_add(xnew[:], x_cols[:], at_ps[:])
            nc.gpsimd.dma_start(xnew_out.ap(), xnew[:])
            h2_bf = small.tile([128, KT], BF16, tag="h2bf")
            rmsnorm(xnew, fn_cols, h2_bf, "2")
            # prefetch the sigmoid table (next function, for silu)
            nc.scalar.activation(warm[:], eps_t[:], AF.Sigmoid)

            # ---------------- ff1 GEMV (one bank, rows 0/32/64) ---------------
            ff1_ps = ps_row.tile([65, 512], F32, tag="row", name="f1ps")
            f1_slots = [
                (ff1_ps[0:1, :], 0, 512),
                (ff1_ps[32:33, :], 512, 1024),
                (ff1_ps[64:65, 0:384], 1024, FFCP),
            ]
            for g in range(8):
                for j in range(4):
                    kt = g * 4 + j
                    st, sp = (kt == 0), (kt == KT - 1)
                    lhs = h2_bf[:, kt : kt + 1]
                    for slot, lo, hi in f1_slots:
                        nc.tensor.matmul(
                            slot, lhs, ff1_c[g][:, j, lo:hi], start=st, stop=sp
                        )

            # ---------------- silu on rows 0/32/64, then PE transposes --------
            sg = work.tile([65, 512], F32, tag="sg")
            nc.scalar.activation(sg[:], ff1_ps[:], AF.Sigmoid)
            silu65 = work.tile([65, 512], F32, tag="silu")
            nc.vector.tensor_mul(silu65[:], sg[:], ff1_ps[:])
            siluT_ps = ps_small.tile([128, FFKT], F32, tag="sm", name="siluTps")
            for t in range(FFKT):
                r, off = (t * 128) // 512, (t * 128) % 512
                b = [0, 32, 64][r]
                nc.tensor.transpose(
                    siluT_ps[:, t : t + 1],
                    silu65[b : b + 1, off : off + 128],
                    ones_col[b : b + 1, 0:1],
                )
            silu_cols = work.tile([128, FFKT], BF16, tag="silu_cols")
            nc.vector.tensor_copy(silu_cols[:], siluT_ps[:])

            # ---------------- ff2 GEMV (8 strips in 3 banks, single pass) -----
            ffrow = work.tile([1, H], F32, tag="ffrow")
            f2_t = [
                ps_row.tile([97, 512], F32, tag="row", name=f"f2ps{i}")
                for i in range(3)
            ]
            f2_slots = [
                f2_t[i // 3][(i % 3) * 32 : (i % 3) * 32 + 1, :] for i in range(8)
            ]
            for kt in range(FFKT):
                lhs = silu_cols[:, kt : kt + 1]
                for i in range(8):
                    nc.tensor.matmul(
                        f2_slots[i],
                        lhs,
                        ff2_c[kt][:, i * 512 : (i + 1) * 512],
                        start=(kt == 0),
                        stop=(kt == FFKT - 1),
                    )
            for i in range(8):
                nc.scalar.copy(ffrow[0:1, i * 512 : (i + 1) * 512], f2_slots[i])
            nc.gpsimd.dma_start(ff_out.ap().rearrange("(p n) -> p n", p=1), ffrow[:])

    nc.compile()
    _BUILD_CACHE[pos] = nc
    return nc


def _pack_k8(w):
    """[4096, N] -> [4, 128, 8, N] tile-major (chunk g, partition p, j, n)."""
    n = w.shape[1]
    return np.ascontiguousarray(
        w.reshape(4, 8, 128, n).transpose(0, 2, 1, 3)
    )


def _shard(inputs, pos):
    f32 = np.float32
    x = np.asarray(inputs["x"], f32)
    an = np.asarray(inputs["attn_norm"], f32)
    fn = np.asarray(inputs["ffn_norm"], f32)
    cos_r = np.asarray(inputs["cos_cache"], f32)[pos]
    sin_r = np.asarray(inputs["sin_cache"], f32)[pos]
    wq = np.asarray(inputs["w_q"], f32)
    wk = np.asarray(inputs["w_k"], f32)
    wv = np.asarray(inputs["w_v"], f32)
    wo = np.asarray(inputs["w_o"], f32)
    kc = np.asarray(inputs["k_cache"], f32)
    vc = np.asarray(inputs["v_cache"], f32)
    ff1 = np.asarray(inputs["w_ff1"], f32)
    ff2 = np.asarray(inputs["w_ff2"], f32)

    def cols(v):
        return np.ascontiguousarray(v.reshape(KT, 128).T)

    x_c, an_c, fn_c = cols(x), cols(an), cols(fn)
    rope_tbl = np.stack(
        [
            np.tile(cos_r, HPC),
            np.tile(sin_r, HPC),
            np.tile(cos_r, HPC) * SCALE,
            np.tile(sin_r, HPC) * SCALE,
        ]
    ).astype(f32)
    eye32 = np.eye(32, dtype=f32)

    in_maps = []
    for c in range(CORES):
        qlo, qhi = c * QC, (c + 1) * QC
        flo, fhi = c * FFC, (c + 1) * FFC
        hlo, hhi = c * HPC, (c + 1) * HPC
        kc_c = kc[:, hlo:hhi, :].reshape(H, QC).astype(NP_BF16)
        vc_c = vc[:, hlo:hhi, :].reshape(H, QC).astype(NP_BF16)
        ff1_c = np.zeros((H, FFCP), dtype=NP_BF16)
        ff1_c[:, :FFC] = ff1[:, flo:fhi].astype(NP_BF16)
        ff2_c = np.zeros((FFKT * 128, H), dtype=NP_BF16)
        ff2_c[:FFC, :] = ff2[flo:fhi, :].astype(NP_BF16)
        in_maps.append(
            {
                "x_cols": x_c,
                "an_cols": an_c,
                "fn_cols": fn_c,
                "rope_tbl": rope_tbl,
                "eye32": eye32,
                "wq": _pack_k8(wq[:, qlo:qhi].astype(NP_BF16)),
                "wk": _pack_k8(wk[:, qlo:qhi].astype(NP_BF16)),
                "wv": _pack_k8(wv[:, qlo:qhi].astype(NP_BF16)),
                "wo": np.ascontiguousarray(
                    wo[qlo:qhi, :].astype(NP_BF16).reshape(4, 128, H)
                ),
                "kc": _pack_k8(kc_c),
                "vc": _pack_k8(vc_c),
                "ff1": np.ascontiguousarray(
                    ff1_c.reshape(8, 4, 128, FFCP).transpose(0, 2, 1, 3)
                ),
                "ff2": np.ascontiguousarray(ff2_c.reshape(FFKT, 128, H)),
            }
        )
    return in_maps


def _assemble(results):
    xnew_cols = results[0]["xnew_out"]  # [128, 32], element (p,t) = vec[t*128+p]
    xnew = np.ascontiguousarray(xnew_cols.T).reshape(-1)
    ff = np.sum(
        np.stack([results[c]["ff_out"] for c in range(CORES)]), axis=0,
        dtype=np.float32,
    )
    return (xnew + ff).astype(np.float32)


def run(inputs, trace=False):
    pos = int(inputs["pos"])
    nc = _build(pos)
    in_maps = _shard(inputs, pos)
    res = run_bass_kernel_spmd(nc, in_maps, core_ids=list(range(CORES)), trace=trace)
    return _assemble(res.results), res


def kernel(**inputs) -> np.ndarray:
    out, _ = run(inputs, trace=False)
    return out
